# revision 24
# baseline (speedup 1.0000x reference)
"""Trainium2 Bass kernel for nn_DoG_Seasonal: depthwise Difference-of-Gaussians
1-D conv along L (sigma 4.2 / 96, reflect padding), y = x*k1 - x*k2.

Multirate scheme (positions on partitions, channels on the free dim):
  - narrow path: k1 (35 taps) exact, on a chunk grid shifted by -17 so each
    output tile of 128 positions needs exactly 2 matmuls: main (chunk m,
    K=128 M=128) + aux (chunk m+1; weight columns 0..93 zero).
  - wide path: k2 (sigma=96) is low-bandwidth -> sketch xc = D x with 128
    overlapping triangular bins (width 64, spacing 32; piecewise-linear row
    space) via 33 "down" matmuls per batch (interior blocks are column-shifted
    slices of ONE [128,248] weight strip), then per tile one "up" matmul
    y2_tile = A_m @ xc (K=128, A_m least-squares fitted per tile against the
    exact reflect k2 operator; minus sign folded in) accumulated into the same
    PSUM bank as the narrow matmuls.

130 matmuls/batch (32 main + 32 aux + 33 down + 32 up + batch-0 warmup
ordering) vs 160 for the dense banded-Toeplitz formulation; evacuation stays
one PSUM->SBUF copy per tile (alternating DVE/ACT). All matmuls are chained
with scheduling-only deps so PSUM accumulate groups execute start-first on
reused banks. Head DMAs are interleaved (narrow w, x pieces, down strip, up
blocks) to minimize the PE cold start; the final output group drains in
2-tile DMA pieces to shorten the tail.

Sharding: data-parallel over batch - 32 batches / 8 cores, no cross-core
communication. Host packs x into a partition-major shifted-chunk DRAM layout
([128, BPC*33*321] bf16) so every DMA is contiguous per partition line;
output is returned the same way and un-packed on host.
"""

import numpy as np
import ml_dtypes

import concourse.bacc as bacc
import concourse.mybir as mybir
import concourse.tile as tile
from concourse.bass_utils import run_bass_kernel_spmd

# ---- problem constants ----
B, L, C = 32, 4096, 321
N_CORES = 8
BPC = B // N_CORES
P = 128
NT = L // P           # 32 output tiles per batch
NCH = NT + 1          # 33 shifted chunks per batch
SH = 17               # chunk grid shift
SIGMA1, SIGMA2, TRUNCATE = 4.2, 96.0, 4.0

import os as _os
OGRP = int(_os.environ.get("DOG_OGRP", "32"))     # out tiles per out-DMA
ACT_EVERY = int(_os.environ.get("DOG_ACT_EVERY", "2"))  # every k-th evac on ScalarE

BF16 = ml_dtypes.bfloat16


# ---------------- host-side weight construction ----------------

def _gauss(sigma):
    r = int(TRUNCATE * sigma + 0.5)
    t = np.arange(-r, r + 1, dtype=np.float64)
    k = np.exp(-0.5 * (t / sigma) ** 2)
    return k / k.sum()


def _chunk_range(c):
    return max(0, 128 * c - SH), min(L, 128 * c - SH + P)


def _reflect(u):
    v = np.abs(u)
    return np.where(v > L - 1, 2 * (L - 1) - v, v)


def _conv_rows(k, rows):
    r = (len(k) - 1) // 2
    W = np.zeros((len(rows), L))
    t = np.arange(-r, r + 1)
    for i, o in enumerate(rows):
        np.add.at(W[i], _reflect(o + t), k)
    return W


def _build_narrow():
    """Per tile m: {chunk c: block [128,128]} (lhsT rows=chunk partitions, cols=outs)."""
    k1 = _gauss(SIGMA1)
    r1 = (len(k1) - 1) // 2
    t = np.arange(-r1, r1 + 1)
    out = []
    for m in range(NT):
        W_full = np.zeros((L, P))
        for j in range(P):
            np.add.at(W_full[:, j], _reflect(128 * m + j + t), k1)
        per = {}
        for c in range(NCH):
            s, e = _chunk_range(c)
            blk = W_full[s:e, :]
            if not np.any(blk):
                continue
            p0 = s - (128 * c - SH)
            full = np.zeros((P, P))
            full[p0:p0 + (e - s), :] = blk
            per[c] = full
        assert set(per) <= {m, m + 1}, (m, sorted(per))
        out.append(per)
    return out


def _build_D():
    """[128 bins, L] overlapping triangular bins (width 64, spacing 32)."""
    D = np.zeros((P, L))
    u = np.arange(L, dtype=np.float64)
    ue = np.clip(u, -1.5, 32.0 * (P - 1) - 1.5)
    for k in range(P):
        D[k] = np.maximum(0.0, 1.0 - np.abs(ue - (32 * k - 1.5)) / 32.0)
    D /= D.sum(axis=0)[None, :]
    D *= 32.0
    return D


def _up_window(m):
    return min(max(4 * m - 14, 0), 96)


def _build_up(D, lam=1e-6):
    """Per tile m: A_m [128, 128] (rows=bins, zero outside the 32-bin window);
    minus sign folded. K=128 contraction of the full xc avoids partition-base
    gymnastics at the cost of per-tile weight blocks."""
    k2 = _gauss(SIGMA2)
    A_int = None
    out = []
    for m in range(NT):
        s = _up_window(m)
        if 4 <= m <= 27 and A_int is not None:
            A32 = A_int
        else:
            K2rows = _conv_rows(k2, np.arange(128 * m, 128 * m + P))
            Dw = D[s:s + 32]
            G = Dw @ Dw.T
            A32 = -np.linalg.solve(G + lam * np.trace(G) / 32 * np.eye(32),
                                   Dw @ K2rows.T)
            if 4 <= m <= 27:
                A_int = A32
        full = np.zeros((P, P))
        full[s:s + 32, :] = A32
        out.append(full)
    return out


def _build_down(D):
    """Down matmuls: list of (chunk c, block [128, 128]). Full M=128 out (zero
    weight columns outside the chunk's ~6-bin support) so the PSUM out AP
    never needs a partition offset."""
    mms = []
    for c in range(NCH):
        s, e = _chunk_range(c)
        p0 = s - (128 * c - SH)
        blk = np.zeros((P, P))           # [chunk partition, bin]
        blk[p0:p0 + (e - s), :] = D[:, s:e].T
        mms.append((c, blk))
    return mms


def _build_weights():
    """Returns (w_np [128, WCOLS] f32, layout dict)."""
    narrow = _build_narrow()
    D = _build_D()
    ups = _build_up(D)
    downs = _build_down(D)

    cols = []          # list of (width, array [128, width])
    uniq = {}

    def intern(arr):
        key = arr.astype(np.float32).tobytes()
        if key not in uniq:
            uniq[key] = (len(uniq), sum(w for w, _ in cols))
            cols.append((arr.shape[1], arr.astype(np.float32)))
        return uniq[key][1]   # column offset

    layout = {"main": [], "aux": [], "down": [], "up": []}
    for m in range(NT):
        layout["main"].append(intern(narrow[m][m]))
        layout["aux"].append(intern(narrow[m][m + 1]))
    # interior down blocks are column-shifted slices of one [128, 248] strip
    dmap = dict(downs)
    Z = 124
    F = np.zeros((P, 248))
    F[:, 60:188] = dmap[16]
    f_off = intern(F)
    for c in range(NCH):
        lo = Z - 4 * c
        if 0 <= lo and lo + P <= 248 and np.allclose(F[:, lo:lo + P], dmap[c], atol=1e-12):
            layout["down"].append((c, f_off + lo))
        else:
            layout["down"].append((c, intern(dmap[c])))
    for m in range(NT):
        layout["up"].append(intern(ups[m]))

    wcols = sum(w for w, _ in cols)
    w_np = np.zeros((P, wcols), np.float32)
    off = 0
    for w, arr in cols:
        w_np[:, off:off + w] = arr
        off += w
    return w_np, layout



def _dedupe_ldweights(nc):
    """Remove redundant InstLdweights: consecutive (in PE program order) loads
    of the identical weights AP need only the first load (~60-107 ns/LDW on HW,
    unmodeled by the cost sim)."""
    removed = 0
    for bi, blk in enumerate(nc.main_func.blocks):
        last_key = None
        new = []
        changed = False
        for inst in blk.instructions:
            nm = type(inst).__name__
            if nm == "InstLdweights":
                key = str(inst.ins[0])
                si = inst.sync_info
                clean = si is None or (len(si.on_wait) == 0 and len(si.on_update) == 0)
                if key == last_key and clean:
                    removed += 1
                    changed = True
                    continue
                last_key = key
            elif nm == "InstMatmult":
                pass
            elif getattr(inst, "engine", None) == mybir.EngineType.PE:
                last_key = None
            new.append(inst)
        if changed:
            blk.instructions = new
    return removed


# ---------------- device program ----------------

def _build_program(wcols, layout):
    _PREV = [None]
    nc = bacc.Bacc(None, target_bir_lowering=False)

    def mm(*args, **kwargs):
        """matmul chained in program order (scheduling-only dep) so PSUM
        accumulate groups execute start-first on reused banks and equal-weight
        runs stay consecutive for LDWEIGHTS."""
        r = nc.tensor.matmul(*args, **kwargs)
        if _PREV[0] is not None:
            tile.add_dep_helper(r.ins, _PREV[0].ins, sync=False, reason="pe order")
        _PREV[0] = r
        return r

    x_d = nc.declare_dram_parameter("x", [P, BPC * NCH * C], mybir.dt.bfloat16, isOutput=False)
    w_d = nc.declare_dram_parameter("w", [P, wcols], mybir.dt.bfloat16, isOutput=False)
    out_d = nc.declare_dram_parameter("out", [P, BPC * NT * C], mybir.dt.bfloat16, isOutput=True)

    with tile.TileContext(nc) as tc:
        with (
            tc.tile_pool(name="wpool", bufs=1) as wpool,
            tc.tile_pool(name="xpool", bufs=int(_os.environ.get("DOG_XBUFS", "2"))) as xpool,
            tc.tile_pool(name="xcsb", bufs=2) as xcsb_pool,
            tc.tile_pool(name="opool", bufs=2) as opool,
            tc.tile_pool(name="psum", bufs=int(_os.environ.get("DOG_PSBUFS", "6")), space="PSUM") as pspool,
            tc.tile_pool(name="xcps", bufs=2, space="PSUM") as xcps_pool,
        ):
            w_sb = wpool.tile([P, wcols], mybir.dt.bfloat16)
            # piece boundaries: narrow blocks | down blocks | up blocks
            wn = min(o for _, o in layout["down"])
            wu = min(layout["up"])
            wu_cuts = [wu, layout["up"][8], layout["up"][16], layout["up"][24], wcols]

            for b in range(BPC):
                xt = xpool.tile([P, NCH, C], mybir.dt.bfloat16, name="xt", tag="xt")

                def xpiece(lo, hi, b=b, xt=xt):
                    nc.sync.dma_start(out=xt[:, lo:hi, :],
                                      in_=x_d[:, (b * NCH + lo) * C:(b * NCH + hi) * C]
                                      .rearrange("p (c n) -> p c n", c=hi - lo))

                if b == 0:
                    # head interleave: wn, x1, wd, x2, x3, wu1, x4, wu2..4
                    nc.sync.dma_start(out=w_sb[:, :wn], in_=w_d[:, :wn])
                    xpiece(0, 7)
                    nc.sync.dma_start(out=w_sb[:, wn:wu], in_=w_d[:, wn:wu])
                    xpiece(7, 17)
                    xpiece(17, 25)
                    xpiece(25, NCH)
                    nc.sync.dma_start(out=w_sb[:, wu_cuts[0]:wu_cuts[1]],
                                      in_=w_d[:, wu_cuts[0]:wu_cuts[1]])
                    for i in range(1, 4):
                        nc.sync.dma_start(out=w_sb[:, wu_cuts[i]:wu_cuts[i + 1]],
                                          in_=w_d[:, wu_cuts[i]:wu_cuts[i + 1]])
                else:
                    for lo, hi in ((0, 9), (9, 17), (17, 25), (25, NCH)):
                        xpiece(lo, hi)

                def mk_psg():
                    return pspool.tile([P, 512], mybir.dt.float32, name="psg", tag="psg")

                def main_aux(m, ps):
                    mm(ps[:, :C], w_sb[:, layout["main"][m]:layout["main"][m] + P],
                       xt[:, m, :], start=True, stop=False)

                def aux(m, ps):
                    mm(ps[:, :C], w_sb[:, layout["aux"][m]:layout["aux"][m] + P],
                       xt[:, m + 1, :], start=False, stop=False)

                def up(m, ps, xc_sb):
                    woff = layout["up"][m]
                    mm(ps[:, :C], w_sb[:, woff:woff + P], xc_sb, start=False, stop=True)

                ogs = {}

                def evac(m, ps, b=b):
                    g = m // OGRP
                    if m % OGRP == 0:
                        ogs[g] = opool.tile([P, OGRP, C], mybir.dt.bfloat16, name="og", tag="og")
                    og = ogs[g]
                    osl = og[:, m % OGRP:m % OGRP + 1, :]
                    if ACT_EVERY and m % ACT_EVERY == ACT_EVERY - 1:
                        nc.scalar.copy(osl, ps[:, None, :C])
                    else:
                        nc.vector.tensor_copy(osl, ps[:, None, :C])
                    last_og = (b == BPC - 1) and (m // OGRP == NT // OGRP - 1)
                    o0 = (m // OGRP) * OGRP
                    dst = lambda i, n: out_d[:, (b * NT + o0 + i) * C:(b * NT + o0 + i + n) * C]
                    if last_og and m % 2 == 1:
                        # tail: drain the final og in 2-tile pieces to overlap DMA
                        i = (m % OGRP) - 1
                        nc.sync.dma_start(out=dst(i, 2).rearrange("p (g n) -> p g n", g=2),
                                          in_=og[:, i:i + 2, :])
                    elif not last_og and m % OGRP == OGRP - 1:
                        nc.sync.dma_start(out=dst(0, OGRP).rearrange("p (g n) -> p g n", g=OGRP),
                                          in_=og)

                WARM = 6 if b == 0 else 0
                warm_ps = {}
                for m in range(WARM):      # batch-0 head: narrow work while w/x stream in
                    warm_ps[m] = mk_psg()
                    main_aux(m, warm_ps[m])
                for m in range(WARM):
                    aux(m, warm_ps[m])

                # ---- down matmuls -> xc psum ----
                xcp = xcps_pool.tile([P, 512], mybir.dt.float32, name="xcp", tag="xcp")
                nd = len(layout["down"])
                for i, (c, off) in enumerate(layout["down"]):
                    mm(xcp[:, :C], w_sb[:, off:off + P], xt[:, c, :],
                       start=(i == 0), stop=(i == nd - 1))
                # ---- xc evac ----
                xc_sb = xcsb_pool.tile([P, C], mybir.dt.bfloat16, name="xc", tag="xc")
                nc.scalar.copy(xc_sb, xcp[:, :C])

                for m in range(WARM):
                    up(m, warm_ps[m], xc_sb)
                    evac(m, warm_ps[m])
                warm_ps = None

                # ---- remaining tiles, processed in pairs for LDW run-sharing ----
                for m0 in range(WARM, NT, 2):
                    pa, pb_ = mk_psg(), mk_psg()
                    main_aux(m0, pa); main_aux(m0 + 1, pb_)
                    aux(m0, pa); aux(m0 + 1, pb_)
                    up(m0, pa, xc_sb); up(m0 + 1, pb_, xc_sb)
                    evac(m0, pa); evac(m0 + 1, pb_)
    _dedupe_ldweights(nc)
    nc.compile()
    return nc


_CACHE = {}


def _get_state():
    if "nc" not in _CACHE:
        w_np, layout = _build_weights()
        _CACHE["w"] = w_np.astype(BF16)
        _CACHE["nc"] = _build_program(w_np.shape[1], layout)
    return _CACHE["nc"], _CACHE["w"]


def _pack_x(xs):
    """xs [BPC, L, C] f32 -> [128, BPC*NCH*C] bf16 (shifted chunk layout)."""
    xb = xs.astype(BF16)
    chunks = np.zeros((BPC, NCH, P, C), BF16)
    chunks[:, 1:32].reshape(BPC, -1, C)[...] = xb[:, 111:4079].reshape(BPC, -1, C)
    chunks[:, 0, SH:, :] = xb[:, :111, :]
    chunks[:, 32, :SH, :] = xb[:, 4079:, :]
    return np.ascontiguousarray(chunks.transpose(2, 0, 1, 3).reshape(P, -1))


def run(x, **spmd_kwargs):
    x = np.asarray(x)
    nc, w_np = _get_state()
    in_maps = [{"x": _pack_x(x[core * BPC:(core + 1) * BPC]), "w": w_np}
               for core in range(N_CORES)]
    res = run_bass_kernel_spmd(nc, in_maps, list(range(N_CORES)), **spmd_kwargs)
    outs = []
    for i in range(N_CORES):
        o = np.asarray(res.results[i]["out"]).reshape(P, BPC, NT, C)
        outs.append(o.transpose(1, 2, 0, 3).reshape(BPC, L, C))
    return np.concatenate(outs, axis=0).astype(np.float32), res


def kernel(x):
    return run(x)[0]


# revision 31
# speedup vs baseline: 1.0053x; 1.0053x over previous
"""Trainium2 Bass kernel for nn_DoG_Seasonal: depthwise Difference-of-Gaussians
1-D conv along L (sigma 4.2 / 96, reflect padding), y = x*k1 - x*k2.

Multirate scheme (positions on partitions, channels on the free dim):
  - narrow path: k1 (35 taps) exact, on a chunk grid shifted by -17 so each
    output tile of 128 positions needs exactly 2 matmuls: main (chunk m,
    K=128 M=128) + aux (chunk m+1; weight columns 0..93 zero).
  - wide path: k2 (sigma=96) is low-bandwidth -> sketch xc = D x with 128
    overlapping triangular bins (width 64, spacing 32; piecewise-linear row
    space) via 33 "down" matmuls per batch (interior blocks are column-shifted
    slices of ONE [128,248] weight strip), then per tile one "up" matmul
    y2_tile = A_m @ xc (K=128, A_m least-squares fitted per tile against the
    exact reflect k2 operator; minus sign folded in) accumulated into the same
    PSUM bank as the narrow matmuls.

130 matmuls/batch (32 main + 32 aux + 33 down + 32 up + batch-0 warmup
ordering) vs 160 for the dense banded-Toeplitz formulation; evacuation stays
one PSUM->SBUF copy per tile (alternating DVE/ACT). All matmuls are chained
with scheduling-only deps so PSUM accumulate groups execute start-first on
reused banks. Head DMAs are interleaved (narrow w, x pieces, down strip, up
blocks) to minimize the PE cold start; the final output group drains in
2-tile DMA pieces to shorten the tail.

Sharding: data-parallel over batch - 32 batches / 8 cores, no cross-core
communication. Host packs x into a partition-major shifted-chunk DRAM layout
([128, BPC*33*321] bf16) so every DMA is contiguous per partition line;
output is returned the same way and un-packed on host.
"""

import numpy as np
import ml_dtypes

import concourse.bacc as bacc
import concourse.mybir as mybir
import concourse.tile as tile
from concourse.bass_utils import run_bass_kernel_spmd

# ---- problem constants ----
B, L, C = 32, 4096, 321
N_CORES = 8
BPC = B // N_CORES
P = 128
NT = L // P           # 32 output tiles per batch
NCH = NT + 1          # 33 shifted chunks per batch
SH = 17               # chunk grid shift
SIGMA1, SIGMA2, TRUNCATE = 4.2, 96.0, 4.0

import os as _os
OGRP = int(_os.environ.get("DOG_OGRP", "32"))     # out tiles per out-DMA
ACT_EVERY = int(_os.environ.get("DOG_ACT_EVERY", "2"))  # every k-th evac on ScalarE

BF16 = ml_dtypes.bfloat16


# ---------------- host-side weight construction ----------------

def _gauss(sigma):
    r = int(TRUNCATE * sigma + 0.5)
    t = np.arange(-r, r + 1, dtype=np.float64)
    k = np.exp(-0.5 * (t / sigma) ** 2)
    return k / k.sum()


def _chunk_range(c):
    return max(0, 128 * c - SH), min(L, 128 * c - SH + P)


def _reflect(u):
    v = np.abs(u)
    return np.where(v > L - 1, 2 * (L - 1) - v, v)


def _conv_rows(k, rows):
    r = (len(k) - 1) // 2
    W = np.zeros((len(rows), L))
    t = np.arange(-r, r + 1)
    for i, o in enumerate(rows):
        np.add.at(W[i], _reflect(o + t), k)
    return W


def _build_narrow():
    """Per tile m: {chunk c: block [128,128]} (lhsT rows=chunk partitions, cols=outs)."""
    k1 = _gauss(SIGMA1)
    r1 = (len(k1) - 1) // 2
    t = np.arange(-r1, r1 + 1)
    out = []
    for m in range(NT):
        W_full = np.zeros((L, P))
        for j in range(P):
            np.add.at(W_full[:, j], _reflect(128 * m + j + t), k1)
        per = {}
        for c in range(NCH):
            s, e = _chunk_range(c)
            blk = W_full[s:e, :]
            if not np.any(blk):
                continue
            p0 = s - (128 * c - SH)
            full = np.zeros((P, P))
            full[p0:p0 + (e - s), :] = blk
            per[c] = full
        assert set(per) <= {m, m + 1}, (m, sorted(per))
        out.append(per)
    return out


def _build_D():
    """[128 bins, L] overlapping triangular bins (width 64, spacing 32)."""
    D = np.zeros((P, L))
    u = np.arange(L, dtype=np.float64)
    ue = np.clip(u, -1.5, 32.0 * (P - 1) - 1.5)
    for k in range(P):
        D[k] = np.maximum(0.0, 1.0 - np.abs(ue - (32 * k - 1.5)) / 32.0)
    D /= D.sum(axis=0)[None, :]
    D *= 32.0
    return D


def _up_window(m):
    return min(max(4 * m - 14, 0), 96)


def _build_up(D, lam=1e-6):
    """Per tile m: A_m [128, 128] (rows=bins, zero outside the 32-bin window);
    minus sign folded. K=128 contraction of the full xc avoids partition-base
    gymnastics at the cost of per-tile weight blocks."""
    k2 = _gauss(SIGMA2)
    A_int = None
    out = []
    for m in range(NT):
        s = _up_window(m)
        if 4 <= m <= 27 and A_int is not None:
            A32 = A_int
        else:
            K2rows = _conv_rows(k2, np.arange(128 * m, 128 * m + P))
            Dw = D[s:s + 32]
            G = Dw @ Dw.T
            A32 = -np.linalg.solve(G + lam * np.trace(G) / 32 * np.eye(32),
                                   Dw @ K2rows.T)
            if 4 <= m <= 27:
                A_int = A32
        full = np.zeros((P, P))
        full[s:s + 32, :] = A32
        out.append(full)
    return out


def _build_down(D):
    """Down matmuls: list of (chunk c, block [128, 128]). Full M=128 out (zero
    weight columns outside the chunk's ~6-bin support) so the PSUM out AP
    never needs a partition offset."""
    mms = []
    for c in range(NCH):
        s, e = _chunk_range(c)
        p0 = s - (128 * c - SH)
        blk = np.zeros((P, P))           # [chunk partition, bin]
        blk[p0:p0 + (e - s), :] = D[:, s:e].T
        mms.append((c, blk))
    return mms


def _build_weights():
    """Returns (w_np [128, WCOLS] f32, layout dict)."""
    narrow = _build_narrow()
    D = _build_D()
    ups = _build_up(D)
    downs = _build_down(D)

    cols = []          # list of (width, array [128, width])
    uniq = {}

    def intern(arr):
        key = arr.astype(np.float32).tobytes()
        if key not in uniq:
            uniq[key] = (len(uniq), sum(w for w, _ in cols))
            cols.append((arr.shape[1], arr.astype(np.float32)))
        return uniq[key][1]   # column offset

    layout = {"main": [], "aux": [], "down": [], "up": []}
    for m in range(NT):
        layout["main"].append(intern(narrow[m][m]))
        layout["aux"].append(intern(narrow[m][m + 1]))
    # interior down blocks are column-shifted slices of one [128, 248] strip
    dmap = dict(downs)
    Z = 124
    F = np.zeros((P, 248))
    F[:, 60:188] = dmap[16]
    f_off = intern(F)
    for c in range(NCH):
        lo = Z - 4 * c
        if 0 <= lo and lo + P <= 248 and np.allclose(F[:, lo:lo + P], dmap[c], atol=1e-12):
            layout["down"].append((c, f_off + lo))
        else:
            layout["down"].append((c, intern(dmap[c])))
    for m in range(NT):
        layout["up"].append(intern(ups[m]))

    wcols = sum(w for w, _ in cols)
    w_np = np.zeros((P, wcols), np.float32)
    off = 0
    for w, arr in cols:
        w_np[:, off:off + w] = arr
        off += w
    return w_np, layout



def _dedupe_ldweights(nc):
    """Remove redundant InstLdweights: consecutive (in PE program order) loads
    of the identical weights AP need only the first load (~60-107 ns/LDW on HW,
    unmodeled by the cost sim)."""
    removed = 0
    for bi, blk in enumerate(nc.main_func.blocks):
        last_key = None
        new = []
        changed = False
        for inst in blk.instructions:
            nm = type(inst).__name__
            if nm == "InstLdweights":
                key = str(inst.ins[0])
                si = inst.sync_info
                clean = si is None or (len(si.on_wait) == 0 and len(si.on_update) == 0)
                if key == last_key and clean:
                    removed += 1
                    changed = True
                    continue
                last_key = key
            elif nm == "InstMatmult":
                pass
            elif getattr(inst, "engine", None) == mybir.EngineType.PE:
                last_key = None
            new.append(inst)
        if changed:
            blk.instructions = new
    return removed


# ---------------- device program ----------------

def _build_program(wcols, layout):
    _PREV = [None]
    nc = bacc.Bacc(None, target_bir_lowering=False)

    def mm(*args, **kwargs):
        """matmul chained in program order (scheduling-only dep) so PSUM
        accumulate groups execute start-first on reused banks and equal-weight
        runs stay consecutive for LDWEIGHTS."""
        r = nc.tensor.matmul(*args, **kwargs)
        if _PREV[0] is not None:
            tile.add_dep_helper(r.ins, _PREV[0].ins, sync=False, reason="pe order")
        _PREV[0] = r
        return r

    x_d = nc.declare_dram_parameter("x", [P, BPC * NCH * C], mybir.dt.bfloat16, isOutput=False)
    w_d = nc.declare_dram_parameter("w", [P, wcols], mybir.dt.bfloat16, isOutput=False)
    out_d = nc.declare_dram_parameter("out", [P, BPC * NT * C], mybir.dt.bfloat16, isOutput=True)

    with tile.TileContext(nc) as tc:
        with (
            tc.tile_pool(name="wpool", bufs=1) as wpool,
            tc.tile_pool(name="xpool", bufs=int(_os.environ.get("DOG_XBUFS", "2"))) as xpool,
            tc.tile_pool(name="xcsb", bufs=2) as xcsb_pool,
            tc.tile_pool(name="opool", bufs=2) as opool,
            tc.tile_pool(name="psum", bufs=int(_os.environ.get("DOG_PSBUFS", "7")), space="PSUM") as pspool,
            tc.tile_pool(name="xcps", bufs=int(_os.environ.get("DOG_XCPSBUFS", "1")), space="PSUM") as xcps_pool,
        ):
            w_sb = wpool.tile([P, wcols], mybir.dt.bfloat16)
            # piece boundaries: narrow blocks | down blocks | up blocks
            wn = min(o for _, o in layout["down"])
            wu = min(layout["up"])
            wu_cuts = [wu, layout["up"][8], layout["up"][16], layout["up"][24], wcols]

            for b in range(BPC):
                xt = xpool.tile([P, NCH, C], mybir.dt.bfloat16, name="xt", tag="xt")

                def xpiece(lo, hi, b=b, xt=xt):
                    nc.sync.dma_start(out=xt[:, lo:hi, :],
                                      in_=x_d[:, (b * NCH + lo) * C:(b * NCH + hi) * C]
                                      .rearrange("p (c n) -> p c n", c=hi - lo))

                if b == 0:
                    # head interleave: wn, x1, wd, x2, x3, wu1, x4, wu2..4
                    nc.sync.dma_start(out=w_sb[:, :wn], in_=w_d[:, :wn])
                    xpiece(0, 7)
                    nc.sync.dma_start(out=w_sb[:, wn:wu], in_=w_d[:, wn:wu])
                    xpiece(7, 17)
                    xpiece(17, 25)
                    xpiece(25, NCH)
                    nc.sync.dma_start(out=w_sb[:, wu_cuts[0]:wu_cuts[1]],
                                      in_=w_d[:, wu_cuts[0]:wu_cuts[1]])
                    for i in range(1, 4):
                        nc.sync.dma_start(out=w_sb[:, wu_cuts[i]:wu_cuts[i + 1]],
                                          in_=w_d[:, wu_cuts[i]:wu_cuts[i + 1]])
                else:
                    for lo, hi in ((0, 9), (9, 17), (17, 25), (25, NCH)):
                        xpiece(lo, hi)

                def mk_psg():
                    return pspool.tile([P, 512], mybir.dt.float32, name="psg", tag="psg")

                def main_aux(m, ps):
                    mm(ps[:, :C], w_sb[:, layout["main"][m]:layout["main"][m] + P],
                       xt[:, m, :], start=True, stop=False)

                def aux(m, ps):
                    mm(ps[:, :C], w_sb[:, layout["aux"][m]:layout["aux"][m] + P],
                       xt[:, m + 1, :], start=False, stop=False)

                def up(m, ps, xc_sb):
                    woff = layout["up"][m]
                    mm(ps[:, :C], w_sb[:, woff:woff + P], xc_sb, start=False, stop=True)

                ogs = {}

                def evac(m, ps, b=b):
                    g = m // OGRP
                    if m % OGRP == 0:
                        ogs[g] = opool.tile([P, OGRP, C], mybir.dt.bfloat16, name="og", tag="og")
                    og = ogs[g]
                    osl = og[:, m % OGRP:m % OGRP + 1, :]
                    if ACT_EVERY and m % ACT_EVERY == ACT_EVERY - 1:
                        nc.scalar.copy(osl, ps[:, None, :C])
                    else:
                        nc.vector.tensor_copy(osl, ps[:, None, :C])
                    last_og = (b == BPC - 1) and (m // OGRP == NT // OGRP - 1)
                    o0 = (m // OGRP) * OGRP
                    dst = lambda i, n: out_d[:, (b * NT + o0 + i) * C:(b * NT + o0 + i + n) * C]
                    if last_og and m % 2 == 1:
                        # tail: drain the final og in 2-tile pieces to overlap DMA
                        i = (m % OGRP) - 1
                        nc.sync.dma_start(out=dst(i, 2).rearrange("p (g n) -> p g n", g=2),
                                          in_=og[:, i:i + 2, :])
                    elif not last_og and m % OGRP == OGRP - 1:
                        nc.sync.dma_start(out=dst(0, OGRP).rearrange("p (g n) -> p g n", g=OGRP),
                                          in_=og)

                WARM = int(_os.environ.get("DOG_WARM", "6")) if b == 0 else 0
                warm_ps = {}
                for m in range(WARM):
                    warm_ps[m] = mk_psg()
                for m in range(WARM):
                    main_aux(m, warm_ps[m])
                for m in range(WARM):
                    aux(m, warm_ps[m])

                # ---- down matmuls -> xc psum ----
                xcp = xcps_pool.tile([P, 512], mybir.dt.float32, name="xcp", tag="xcp")
                nd = len(layout["down"])
                for i, (c, off) in enumerate(layout["down"]):
                    mm(xcp[:, :C], w_sb[:, off:off + P], xt[:, c, :],
                       start=(i == 0), stop=(i == nd - 1))
                # ---- xc evac ----
                xc_sb = xcsb_pool.tile([P, C], mybir.dt.bfloat16, name="xc", tag="xc")
                nc.vector.tensor_copy(xc_sb, xcp[:, :C])

                for m in range(WARM):
                    up(m, warm_ps[m], xc_sb)
                    evac(m, warm_ps[m])
                warm_ps = None

                # ---- remaining tiles, processed in pairs for LDW run-sharing ----
                for m0 in range(WARM, NT, 2):
                    pa, pb_ = mk_psg(), mk_psg()
                    main_aux(m0, pa); main_aux(m0 + 1, pb_)
                    aux(m0, pa); aux(m0 + 1, pb_)
                    up(m0, pa, xc_sb); up(m0 + 1, pb_, xc_sb)
                    evac(m0, pa); evac(m0 + 1, pb_)
    _dedupe_ldweights(nc)
    nc.compile()
    return nc


_CACHE = {}


def _get_state():
    if "nc" not in _CACHE:
        w_np, layout = _build_weights()
        _CACHE["w"] = w_np.astype(BF16)
        _CACHE["nc"] = _build_program(w_np.shape[1], layout)
    return _CACHE["nc"], _CACHE["w"]


def _pack_x(xs):
    """xs [BPC, L, C] f32 -> [128, BPC*NCH*C] bf16 (shifted chunk layout)."""
    xb = xs.astype(BF16)
    chunks = np.zeros((BPC, NCH, P, C), BF16)
    chunks[:, 1:32].reshape(BPC, -1, C)[...] = xb[:, 111:4079].reshape(BPC, -1, C)
    chunks[:, 0, SH:, :] = xb[:, :111, :]
    chunks[:, 32, :SH, :] = xb[:, 4079:, :]
    return np.ascontiguousarray(chunks.transpose(2, 0, 1, 3).reshape(P, -1))


def run(x, **spmd_kwargs):
    x = np.asarray(x)
    nc, w_np = _get_state()
    in_maps = [{"x": _pack_x(x[core * BPC:(core + 1) * BPC]), "w": w_np}
               for core in range(N_CORES)]
    res = run_bass_kernel_spmd(nc, in_maps, list(range(N_CORES)), **spmd_kwargs)
    outs = []
    for i in range(N_CORES):
        o = np.asarray(res.results[i]["out"]).reshape(P, BPC, NT, C)
        outs.append(o.transpose(1, 2, 0, 3).reshape(BPC, L, C))
    return np.concatenate(outs, axis=0).astype(np.float32), res


def kernel(x):
    return run(x)[0]


# revision 37
# speedup vs baseline: 1.0074x; 1.0021x over previous
"""Trainium2 Bass kernel for nn_DoG_Seasonal: depthwise Difference-of-Gaussians
1-D conv along L (sigma 4.2 / 96, reflect padding), y = x*k1 - x*k2.

Multirate scheme (positions on partitions, channels on the free dim):
  - narrow path: k1 (35 taps) exact, on a chunk grid shifted by -17 so each
    output tile of 128 positions needs exactly 2 matmuls: main (chunk m,
    K=128 M=128) + aux (chunk m+1; weight columns 0..93 zero).
  - wide path: k2 (sigma=96) is low-bandwidth -> sketch xc = D x with 128
    overlapping triangular bins (width 64, spacing 32; piecewise-linear row
    space) via 33 "down" matmuls per batch (interior blocks are column-shifted
    slices of ONE [128,248] weight strip), then per tile one "up" matmul
    y2_tile = A_m @ xc (K=128, A_m least-squares fitted per tile against the
    exact reflect k2 operator; minus sign folded in) accumulated into the same
    PSUM bank as the narrow matmuls.

130 matmuls/batch (32 main + 32 aux + 33 down + 32 up + batch-0 warmup
ordering) vs 160 for the dense banded-Toeplitz formulation; evacuation stays
one PSUM->SBUF copy per tile (alternating DVE/ACT). All matmuls are chained
with scheduling-only deps so PSUM accumulate groups execute start-first on
reused banks. Head DMAs are interleaved (narrow w, x pieces, down strip, up
blocks) to minimize the PE cold start; the final output group drains in
2-tile DMA pieces to shorten the tail.

Sharding: data-parallel over batch - 32 batches / 8 cores, no cross-core
communication. Host packs x into a partition-major shifted-chunk DRAM layout
([128, BPC*33*321] bf16) so every DMA is contiguous per partition line;
output is returned the same way and un-packed on host.
"""

import numpy as np
import ml_dtypes

import concourse.bacc as bacc
import concourse.mybir as mybir
import concourse.tile as tile
from concourse.bass_utils import run_bass_kernel_spmd

# ---- problem constants ----
B, L, C = 32, 4096, 321
N_CORES = 8
BPC = B // N_CORES
P = 128
NT = L // P           # 32 output tiles per batch
NCH = NT + 1          # 33 shifted chunks per batch
SH = 17               # chunk grid shift
SIGMA1, SIGMA2, TRUNCATE = 4.2, 96.0, 4.0

import os as _os
OGRP = int(_os.environ.get("DOG_OGRP", "32"))     # out tiles per out-DMA
ACT_EVERY = int(_os.environ.get("DOG_ACT_EVERY", "2"))  # every k-th evac on ScalarE

BF16 = ml_dtypes.bfloat16


# ---------------- host-side weight construction ----------------

def _gauss(sigma):
    r = int(TRUNCATE * sigma + 0.5)
    t = np.arange(-r, r + 1, dtype=np.float64)
    k = np.exp(-0.5 * (t / sigma) ** 2)
    return k / k.sum()


def _chunk_range(c):
    return max(0, 128 * c - SH), min(L, 128 * c - SH + P)


def _reflect(u):
    v = np.abs(u)
    return np.where(v > L - 1, 2 * (L - 1) - v, v)


def _conv_rows(k, rows):
    r = (len(k) - 1) // 2
    W = np.zeros((len(rows), L))
    t = np.arange(-r, r + 1)
    for i, o in enumerate(rows):
        np.add.at(W[i], _reflect(o + t), k)
    return W


def _build_narrow():
    """Per tile m: {chunk c: block [128,128]} (lhsT rows=chunk partitions, cols=outs)."""
    k1 = _gauss(SIGMA1)
    r1 = (len(k1) - 1) // 2
    t = np.arange(-r1, r1 + 1)
    out = []
    for m in range(NT):
        W_full = np.zeros((L, P))
        for j in range(P):
            np.add.at(W_full[:, j], _reflect(128 * m + j + t), k1)
        per = {}
        for c in range(NCH):
            s, e = _chunk_range(c)
            blk = W_full[s:e, :]
            if not np.any(blk):
                continue
            p0 = s - (128 * c - SH)
            full = np.zeros((P, P))
            full[p0:p0 + (e - s), :] = blk
            per[c] = full
        assert set(per) <= {m, m + 1}, (m, sorted(per))
        out.append(per)
    return out


def _build_D():
    """[128 bins, L] overlapping triangular bins (width 64, spacing 32)."""
    D = np.zeros((P, L))
    u = np.arange(L, dtype=np.float64)
    ue = np.clip(u, -1.5, 32.0 * (P - 1) - 1.5)
    for k in range(P):
        D[k] = np.maximum(0.0, 1.0 - np.abs(ue - (32 * k - 1.5)) / 32.0)
    D /= D.sum(axis=0)[None, :]
    D *= 32.0
    return D


def _up_window(m):
    return min(max(4 * m - 14, 0), 96)


def _build_up(D, lam=1e-6):
    """Per tile m: A_m [128, 128] (rows=bins, zero outside the 32-bin window);
    minus sign folded. K=128 contraction of the full xc avoids partition-base
    gymnastics at the cost of per-tile weight blocks."""
    k2 = _gauss(SIGMA2)
    A_int = None
    out = []
    for m in range(NT):
        s = _up_window(m)
        if 4 <= m <= 27 and A_int is not None:
            A32 = A_int
        else:
            K2rows = _conv_rows(k2, np.arange(128 * m, 128 * m + P))
            Dw = D[s:s + 32]
            G = Dw @ Dw.T
            A32 = -np.linalg.solve(G + lam * np.trace(G) / 32 * np.eye(32),
                                   Dw @ K2rows.T)
            if 4 <= m <= 27:
                A_int = A32
        full = np.zeros((P, P))
        full[s:s + 32, :] = A32
        out.append(full)
    return out


def _build_down(D):
    """Down matmuls: list of (chunk c, block [128, 128]). Full M=128 out (zero
    weight columns outside the chunk's ~6-bin support) so the PSUM out AP
    never needs a partition offset."""
    mms = []
    for c in range(NCH):
        s, e = _chunk_range(c)
        p0 = s - (128 * c - SH)
        blk = np.zeros((P, P))           # [chunk partition, bin]
        blk[p0:p0 + (e - s), :] = D[:, s:e].T
        mms.append((c, blk))
    return mms


def _build_weights():
    """Returns (w_np [128, WCOLS] f32, layout dict)."""
    narrow = _build_narrow()
    D = _build_D()
    ups = _build_up(D)
    downs = _build_down(D)

    cols = []          # list of (width, array [128, width])
    uniq = {}

    def intern(arr):
        key = arr.astype(np.float32).tobytes()
        if key not in uniq:
            uniq[key] = (len(uniq), sum(w for w, _ in cols))
            cols.append((arr.shape[1], arr.astype(np.float32)))
        return uniq[key][1]   # column offset

    layout = {"main": [], "aux": [], "down": [], "up": []}
    for m in range(NT):
        layout["main"].append(intern(narrow[m][m]))
        layout["aux"].append(intern(narrow[m][m + 1]))
    # interior down blocks are column-shifted slices of one [128, 248] strip
    dmap = dict(downs)
    Z = 124
    F = np.zeros((P, 248))
    F[:, 60:188] = dmap[16]
    f_off = intern(F)
    for c in range(NCH):
        lo = Z - 4 * c
        if 0 <= lo and lo + P <= 248 and np.allclose(F[:, lo:lo + P], dmap[c], atol=1e-12):
            layout["down"].append((c, f_off + lo))
        else:
            layout["down"].append((c, intern(dmap[c])))
    for m in range(NT):
        layout["up"].append(intern(ups[m]))

    wcols = sum(w for w, _ in cols)
    w_np = np.zeros((P, wcols), np.float32)
    off = 0
    for w, arr in cols:
        w_np[:, off:off + w] = arr
        off += w
    return w_np, layout



def _dedupe_ldweights(nc):
    """Remove redundant InstLdweights: consecutive (in PE program order) loads
    of the identical weights AP need only the first load (~60-107 ns/LDW on HW,
    unmodeled by the cost sim)."""
    removed = 0
    for bi, blk in enumerate(nc.main_func.blocks):
        last_key = None
        new = []
        changed = False
        for inst in blk.instructions:
            nm = type(inst).__name__
            if nm == "InstLdweights":
                key = str(inst.ins[0])
                si = inst.sync_info
                clean = si is None or (len(si.on_wait) == 0 and len(si.on_update) == 0)
                if key == last_key and clean:
                    removed += 1
                    changed = True
                    continue
                last_key = key
            elif nm == "InstMatmult":
                pass
            elif getattr(inst, "engine", None) == mybir.EngineType.PE:
                last_key = None
            new.append(inst)
        if changed:
            blk.instructions = new
    return removed


# ---------------- device program ----------------

def _build_program(wcols, layout):
    _PREV = [None]
    nc = bacc.Bacc(None, target_bir_lowering=False)

    def mm(*args, **kwargs):
        """matmul chained in program order (scheduling-only dep) so PSUM
        accumulate groups execute start-first on reused banks and equal-weight
        runs stay consecutive for LDWEIGHTS."""
        r = nc.tensor.matmul(*args, **kwargs)
        if _PREV[0] is not None:
            tile.add_dep_helper(r.ins, _PREV[0].ins, sync=False, reason="pe order")
        _PREV[0] = r
        return r

    x_d = nc.declare_dram_parameter("x", [P, BPC * NCH * C], mybir.dt.bfloat16, isOutput=False)
    w_d = nc.declare_dram_parameter("w", [P, wcols], mybir.dt.bfloat16, isOutput=False)
    out_d = nc.declare_dram_parameter("out", [P, BPC * NT * C], mybir.dt.bfloat16, isOutput=True)

    with tile.TileContext(nc) as tc:
        with (
            tc.tile_pool(name="wpool", bufs=1) as wpool,
            tc.tile_pool(name="xpool", bufs=int(_os.environ.get("DOG_XBUFS", "2"))) as xpool,
            tc.tile_pool(name="xcsb", bufs=2) as xcsb_pool,
            tc.tile_pool(name="opool", bufs=2) as opool,
            tc.tile_pool(name="psum", bufs=int(_os.environ.get("DOG_PSBUFS", "7")), space="PSUM") as pspool,
            tc.tile_pool(name="xcps", bufs=int(_os.environ.get("DOG_XCPSBUFS", "1")), space="PSUM") as xcps_pool,
        ):
            w_sb = wpool.tile([P, wcols], mybir.dt.bfloat16)
            # piece boundaries: narrow blocks | down blocks | up blocks
            wn = min(o for _, o in layout["down"])
            wu = min(layout["up"])
            wu_cuts = [wu, layout["up"][8], layout["up"][16], layout["up"][24], wcols]
            # up blocks have only 32 nonzero rows; zero the region once (DVE is
            # idle at the head) and DMA just the nonzero row-bands per quad
            nc.vector.memset(w_sb[:, wu:], 0.0)
            wu_rows = []
            for q in range(4):
                r0 = _up_window(8 * q)
                r1 = _up_window(8 * q + 7) + 32
                wu_rows.append((r0, r1))

            for b in range(BPC):
                xt = xpool.tile([P, NCH, C], mybir.dt.bfloat16, name="xt", tag="xt")

                def xpiece(lo, hi, b=b, xt=xt):
                    nc.sync.dma_start(out=xt[:, lo:hi, :],
                                      in_=x_d[:, (b * NCH + lo) * C:(b * NCH + hi) * C]
                                      .rearrange("p (c n) -> p c n", c=hi - lo))

                if b == 0:
                    # head interleave: wn, x1, wd, x2, x3, wu1, x4, wu2..4
                    nc.sync.dma_start(out=w_sb[:, :wn], in_=w_d[:, :wn])
                    xpiece(0, 3)
                    xpiece(3, 7)
                    nc.sync.dma_start(out=w_sb[:, wn:wu], in_=w_d[:, wn:wu])
                    xpiece(7, 12)
                    xpiece(12, 17)
                    xpiece(17, 22)
                    xpiece(22, 27)
                    xpiece(27, NCH)
                    for i in range(4):
                        r0, r1 = wu_rows[i]
                        nc.sync.dma_start(
                            out=w_sb[r0:r1, wu_cuts[i]:wu_cuts[i + 1]],
                            in_=w_d[r0:r1, wu_cuts[i]:wu_cuts[i + 1]])
                else:
                    for lo, hi in ((0, 9), (9, 17), (17, 25), (25, NCH)):
                        xpiece(lo, hi)

                def mk_psg():
                    return pspool.tile([P, 512], mybir.dt.float32, name="psg", tag="psg")

                def main_aux(m, ps):
                    mm(ps[:, :C], w_sb[:, layout["main"][m]:layout["main"][m] + P],
                       xt[:, m, :], start=True, stop=False)

                def aux(m, ps):
                    mm(ps[:, :C], w_sb[:, layout["aux"][m]:layout["aux"][m] + P],
                       xt[:, m + 1, :], start=False, stop=False)

                def up(m, ps, xc_sb):
                    woff = layout["up"][m]
                    mm(ps[:, :C], w_sb[:, woff:woff + P], xc_sb, start=False, stop=True)

                ogs = {}

                def evac(m, ps, b=b):
                    g = m // OGRP
                    if m % OGRP == 0:
                        ogs[g] = opool.tile([P, OGRP, C], mybir.dt.bfloat16, name="og", tag="og")
                    og = ogs[g]
                    osl = og[:, m % OGRP:m % OGRP + 1, :]
                    if ACT_EVERY and m % ACT_EVERY == ACT_EVERY - 1:
                        nc.scalar.copy(osl, ps[:, None, :C])
                    else:
                        nc.vector.tensor_copy(osl, ps[:, None, :C])
                    last_og = (b == BPC - 1) and (m // OGRP == NT // OGRP - 1)
                    o0 = (m // OGRP) * OGRP
                    dst = lambda i, n: out_d[:, (b * NT + o0 + i) * C:(b * NT + o0 + i + n) * C]
                    if last_og and m % 2 == 1:
                        # tail: drain the final og in 2-tile pieces to overlap DMA
                        i = (m % OGRP) - 1
                        nc.sync.dma_start(out=dst(i, 2).rearrange("p (g n) -> p g n", g=2),
                                          in_=og[:, i:i + 2, :])
                    elif not last_og and m % OGRP == OGRP - 1:
                        nc.sync.dma_start(out=dst(0, OGRP).rearrange("p (g n) -> p g n", g=OGRP),
                                          in_=og)

                WARM = int(_os.environ.get("DOG_WARM", "6")) if b == 0 else 0
                warm_ps = {}
                for m in range(WARM):
                    warm_ps[m] = mk_psg()
                for m in range(WARM):
                    main_aux(m, warm_ps[m])
                for m in range(WARM):
                    aux(m, warm_ps[m])

                # ---- down matmuls -> xc psum ----
                xcp = xcps_pool.tile([P, 512], mybir.dt.float32, name="xcp", tag="xcp")
                nd = len(layout["down"])
                for i, (c, off) in enumerate(layout["down"]):
                    mm(xcp[:, :C], w_sb[:, off:off + P], xt[:, c, :],
                       start=(i == 0), stop=(i == nd - 1))
                # ---- xc evac ----
                xc_sb = xcsb_pool.tile([P, C], mybir.dt.bfloat16, name="xc", tag="xc")
                nc.vector.tensor_copy(xc_sb, xcp[:, :C])

                for m in range(WARM):
                    up(m, warm_ps[m], xc_sb)
                    evac(m, warm_ps[m])
                warm_ps = None

                # ---- remaining tiles, processed in pairs for LDW run-sharing ----
                for m0 in range(WARM, NT, 2):
                    pa, pb_ = mk_psg(), mk_psg()
                    main_aux(m0, pa); main_aux(m0 + 1, pb_)
                    aux(m0, pa); aux(m0 + 1, pb_)
                    up(m0, pa, xc_sb); up(m0 + 1, pb_, xc_sb)
                    evac(m0, pa); evac(m0 + 1, pb_)
    _dedupe_ldweights(nc)
    nc.compile()
    return nc


_CACHE = {}


def _get_state():
    if "nc" not in _CACHE:
        w_np, layout = _build_weights()
        _CACHE["w"] = w_np.astype(BF16)
        _CACHE["nc"] = _build_program(w_np.shape[1], layout)
    return _CACHE["nc"], _CACHE["w"]


def _pack_x(xs):
    """xs [BPC, L, C] f32 -> [128, BPC*NCH*C] bf16 (shifted chunk layout)."""
    xb = xs.astype(BF16)
    chunks = np.zeros((BPC, NCH, P, C), BF16)
    chunks[:, 1:32].reshape(BPC, -1, C)[...] = xb[:, 111:4079].reshape(BPC, -1, C)
    chunks[:, 0, SH:, :] = xb[:, :111, :]
    chunks[:, 32, :SH, :] = xb[:, 4079:, :]
    return np.ascontiguousarray(chunks.transpose(2, 0, 1, 3).reshape(P, -1))


def run(x, **spmd_kwargs):
    x = np.asarray(x)
    nc, w_np = _get_state()
    in_maps = [{"x": _pack_x(x[core * BPC:(core + 1) * BPC]), "w": w_np}
               for core in range(N_CORES)]
    res = run_bass_kernel_spmd(nc, in_maps, list(range(N_CORES)), **spmd_kwargs)
    outs = []
    for i in range(N_CORES):
        o = np.asarray(res.results[i]["out"]).reshape(P, BPC, NT, C)
        outs.append(o.transpose(1, 2, 0, 3).reshape(BPC, L, C))
    return np.concatenate(outs, axis=0).astype(np.float32), res


def kernel(x):
    return run(x)[0]


# revision 41
# speedup vs baseline: 1.0188x; 1.0113x over previous
"""Trainium2 Bass kernel for nn_DoG_Seasonal: depthwise Difference-of-Gaussians
1-D conv along L (sigma 4.2 / 96, reflect padding), y = x*k1 - x*k2.

Multirate scheme (positions on partitions, channels on the free dim):
  - narrow path: k1 (35 taps) exact, on a chunk grid shifted by -17 so each
    output tile of 128 positions needs exactly 2 matmuls: main (chunk m,
    K=128 M=128) + aux (chunk m+1; weight columns 0..93 zero).
  - wide path: k2 (sigma=96) is low-bandwidth -> sketch xc = D x with 128
    overlapping triangular bins (width 64, spacing 32; piecewise-linear row
    space) via 33 "down" matmuls per batch (interior blocks are column-shifted
    slices of ONE [128,248] weight strip), then per tile one "up" matmul
    y2_tile = A_m @ xc (K=128, A_m least-squares fitted per tile against the
    exact reflect k2 operator; minus sign folded in) accumulated into the same
    PSUM bank as the narrow matmuls.

130 matmuls/batch (32 main + 32 aux + 33 down + 32 up + batch-0 warmup
ordering) vs 160 for the dense banded-Toeplitz formulation; evacuation stays
one PSUM->SBUF copy per tile (alternating DVE/ACT). All matmuls are chained
with scheduling-only deps so PSUM accumulate groups execute start-first on
reused banks. Head DMAs are interleaved (narrow w, x pieces, down strip, up
blocks) to minimize the PE cold start; the final output group drains in
2-tile DMA pieces to shorten the tail.

Sharding: data-parallel over batch - 32 batches / 8 cores, no cross-core
communication. Host packs x into a partition-major shifted-chunk DRAM layout
([128, BPC*33*321] bf16) so every DMA is contiguous per partition line;
output is returned the same way and un-packed on host.
"""

import numpy as np
import ml_dtypes

import concourse.bacc as bacc
import concourse.mybir as mybir
import concourse.tile as tile
from concourse.bass_utils import run_bass_kernel_spmd

# ---- problem constants ----
B, L, C = 32, 4096, 321
N_CORES = 8
BPC = B // N_CORES
P = 128
NT = L // P           # 32 output tiles per batch
NCH = NT + 1          # 33 shifted chunks per batch (conceptual)
NSLOT = NT            # 32 x slots: chunk 32's 17 rows ride in chunk 0's padding
SH = 17               # chunk grid shift
SIGMA1, SIGMA2, TRUNCATE = 4.2, 96.0, 4.0

import os as _os
OGRP = int(_os.environ.get("DOG_OGRP", "32"))     # out tiles per out-DMA
ACT_EVERY = int(_os.environ.get("DOG_ACT_EVERY", "2"))  # every k-th evac on ScalarE

BF16 = ml_dtypes.bfloat16


# ---------------- host-side weight construction ----------------

def _gauss(sigma):
    r = int(TRUNCATE * sigma + 0.5)
    t = np.arange(-r, r + 1, dtype=np.float64)
    k = np.exp(-0.5 * (t / sigma) ** 2)
    return k / k.sum()


def _chunk_range(c):
    return max(0, 128 * c - SH), min(L, 128 * c - SH + P)


def _reflect(u):
    v = np.abs(u)
    return np.where(v > L - 1, 2 * (L - 1) - v, v)


def _conv_rows(k, rows):
    r = (len(k) - 1) // 2
    W = np.zeros((len(rows), L))
    t = np.arange(-r, r + 1)
    for i, o in enumerate(rows):
        np.add.at(W[i], _reflect(o + t), k)
    return W


def _build_narrow():
    """Per tile m: {chunk c: block [128,128]} (lhsT rows=chunk partitions, cols=outs)."""
    k1 = _gauss(SIGMA1)
    r1 = (len(k1) - 1) // 2
    t = np.arange(-r1, r1 + 1)
    out = []
    for m in range(NT):
        W_full = np.zeros((L, P))
        for j in range(P):
            np.add.at(W_full[:, j], _reflect(128 * m + j + t), k1)
        per = {}
        for c in range(NCH):
            s, e = _chunk_range(c)
            blk = W_full[s:e, :]
            if not np.any(blk):
                continue
            p0 = s - (128 * c - SH)
            full = np.zeros((P, P))
            full[p0:p0 + (e - s), :] = blk
            per[c] = full
        assert set(per) <= {m, m + 1}, (m, sorted(per))
        out.append(per)
    return out


def _build_D():
    """[128 bins, L] overlapping triangular bins (width 64, spacing 32)."""
    D = np.zeros((P, L))
    u = np.arange(L, dtype=np.float64)
    ue = np.clip(u, -1.5, 32.0 * (P - 1) - 1.5)
    for k in range(P):
        D[k] = np.maximum(0.0, 1.0 - np.abs(ue - (32 * k - 1.5)) / 32.0)
    D /= D.sum(axis=0)[None, :]
    D *= 32.0
    return D


def _up_window(m):
    return min(max(4 * m - 14, 0), 96)


def _build_up(D, lam=1e-6):
    """Per tile m: A_m [128, 128] (rows=bins, zero outside the 32-bin window);
    minus sign folded. K=128 contraction of the full xc avoids partition-base
    gymnastics at the cost of per-tile weight blocks."""
    k2 = _gauss(SIGMA2)
    A_int = None
    out = []
    for m in range(NT):
        s = _up_window(m)
        if 4 <= m <= 27 and A_int is not None:
            A32 = A_int
        else:
            K2rows = _conv_rows(k2, np.arange(128 * m, 128 * m + P))
            Dw = D[s:s + 32]
            G = Dw @ Dw.T
            A32 = -np.linalg.solve(G + lam * np.trace(G) / 32 * np.eye(32),
                                   Dw @ K2rows.T)
            if 4 <= m <= 27:
                A_int = A32
        full = np.zeros((P, P))
        full[s:s + 32, :] = A32
        out.append(full)
    return out


def _build_down(D):
    """Down matmuls: list of (chunk c, block [128, 128]). Full M=128 out (zero
    weight columns outside the chunk's ~6-bin support) so the PSUM out AP
    never needs a partition offset."""
    mms = []
    for c in range(NCH):
        s, e = _chunk_range(c)
        p0 = s - (128 * c - SH)
        blk = np.zeros((P, P))           # [chunk partition, bin]
        blk[p0:p0 + (e - s), :] = D[:, s:e].T
        mms.append((c, blk))
    return mms


def _build_weights():
    """Returns (w_np [128, WCOLS] f32, layout dict)."""
    narrow = _build_narrow()
    D = _build_D()
    ups = _build_up(D)
    downs = _build_down(D)

    cols = []          # list of (width, array [128, width])
    uniq = {}

    def intern(arr):
        key = arr.astype(np.float32).tobytes()
        if key not in uniq:
            uniq[key] = (len(uniq), sum(w for w, _ in cols))
            cols.append((arr.shape[1], arr.astype(np.float32)))
        return uniq[key][1]   # column offset

    layout = {"main": [], "aux": [], "down": [], "up": []}
    for m in range(NT):
        layout["main"].append(intern(narrow[m][m]))
        layout["aux"].append(intern(narrow[m][m + 1]))
    # interior down blocks are column-shifted slices of one [128, 248] strip
    dmap = dict(downs)
    # chunk 32 (17 rows) is packed into chunk-0's zero partitions 0..16, so
    # its down block merges into chunk 0's (disjoint nonzero rows)
    assert not np.any(dmap[0][0:17]) and not np.any(dmap[32][17:])
    dmap[0] = dmap[0] + dmap[32]
    del dmap[32]
    Z = 124
    F = np.zeros((P, 248))
    F[:, 60:188] = dmap[16]
    f_off = intern(F)
    for c in range(NCH - 1):
        lo = Z - 4 * c
        if 0 <= lo and lo + P <= 248 and np.allclose(F[:, lo:lo + P], dmap[c], atol=1e-12):
            layout["down"].append((c, f_off + lo))
        else:
            layout["down"].append((c, intern(dmap[c])))
    for m in range(NT):
        layout["up"].append(intern(ups[m]))

    wcols = sum(w for w, _ in cols)
    w_np = np.zeros((P, wcols), np.float32)
    off = 0
    for w, arr in cols:
        w_np[:, off:off + w] = arr
        off += w
    return w_np, layout



def _dedupe_ldweights(nc):
    """Remove redundant InstLdweights: consecutive (in PE program order) loads
    of the identical weights AP need only the first load (~60-107 ns/LDW on HW,
    unmodeled by the cost sim)."""
    removed = 0
    for bi, blk in enumerate(nc.main_func.blocks):
        last_key = None
        new = []
        changed = False
        for inst in blk.instructions:
            nm = type(inst).__name__
            if nm == "InstLdweights":
                key = str(inst.ins[0])
                si = inst.sync_info
                clean = si is None or (len(si.on_wait) == 0 and len(si.on_update) == 0)
                if key == last_key and clean:
                    removed += 1
                    changed = True
                    continue
                last_key = key
            elif nm == "InstMatmult":
                pass
            elif getattr(inst, "engine", None) == mybir.EngineType.PE:
                last_key = None
            new.append(inst)
        if changed:
            blk.instructions = new
    return removed


# ---------------- device program ----------------

def _build_program(wcols, layout):
    _PREV = [None]
    nc = bacc.Bacc(None, target_bir_lowering=False)

    def mm(*args, **kwargs):
        """matmul chained in program order (scheduling-only dep) so PSUM
        accumulate groups execute start-first on reused banks and equal-weight
        runs stay consecutive for LDWEIGHTS."""
        r = nc.tensor.matmul(*args, **kwargs)
        if _PREV[0] is not None:
            tile.add_dep_helper(r.ins, _PREV[0].ins, sync=False, reason="pe order")
        _PREV[0] = r
        return r

    x_d = nc.declare_dram_parameter("x", [P, BPC * NSLOT * C], mybir.dt.bfloat16, isOutput=False)
    w_d = nc.declare_dram_parameter("w", [P, wcols], mybir.dt.bfloat16, isOutput=False)
    out_d = nc.declare_dram_parameter("out", [P, BPC * NT * C], mybir.dt.bfloat16, isOutput=True)

    with tile.TileContext(nc) as tc:
        with (
            tc.tile_pool(name="wpool", bufs=1) as wpool,
            tc.tile_pool(name="xpool", bufs=int(_os.environ.get("DOG_XBUFS", "2"))) as xpool,
            tc.tile_pool(name="xcsb", bufs=2) as xcsb_pool,
            tc.tile_pool(name="opool", bufs=2) as opool,
            tc.tile_pool(name="psum", bufs=int(_os.environ.get("DOG_PSBUFS", "7")), space="PSUM") as pspool,
            tc.tile_pool(name="xcps", bufs=int(_os.environ.get("DOG_XCPSBUFS", "1")), space="PSUM") as xcps_pool,
        ):
            w_sb = wpool.tile([P, wcols], mybir.dt.bfloat16)
            # piece boundaries: narrow blocks | down blocks | up blocks
            wn = min(o for _, o in layout["down"])
            wu = min(layout["up"])
            wu_cuts = [wu, layout["up"][8], layout["up"][16], layout["up"][24], wcols]
            # up blocks have only 32 nonzero rows; zero the region once (DVE is
            # idle at the head) and DMA just the nonzero row-bands per quad
            nc.vector.memset(w_sb[:, wu:], 0.0)
            wu_rows = []
            for q in range(4):
                r0 = _up_window(8 * q)
                r1 = _up_window(8 * q + 7) + 32
                wu_rows.append((r0, r1))

            for b in range(BPC):
                xt = xpool.tile([P, NSLOT, C], mybir.dt.bfloat16, name="xt", tag="xt")

                def xpiece(lo, hi, b=b, xt=xt):
                    nc.sync.dma_start(out=xt[:, lo:hi, :],
                                      in_=x_d[:, (b * NSLOT + lo) * C:(b * NSLOT + hi) * C]
                                      .rearrange("p (c n) -> p c n", c=hi - lo))

                if b == 0:
                    # head interleave: wn, x1, wd, x2, x3, wu1, x4, wu2..4
                    nc.sync.dma_start(out=w_sb[:, :wn], in_=w_d[:, :wn])
                    xpiece(0, 3)
                    xpiece(3, 7)
                    nc.sync.dma_start(out=w_sb[:, wn:wu], in_=w_d[:, wn:wu])
                    xpiece(7, 11)
                    xpiece(11, 15)
                    xpiece(15, 19)
                    xpiece(19, 23)
                    xpiece(23, 27)
                    xpiece(27, NSLOT)
                    for i in range(4):
                        r0, r1 = wu_rows[i]
                        nc.sync.dma_start(
                            out=w_sb[r0:r1, wu_cuts[i]:wu_cuts[i + 1]],
                            in_=w_d[r0:r1, wu_cuts[i]:wu_cuts[i + 1]])
                else:
                    for lo, hi in ((0, 9), (9, 17), (17, 25), (25, NSLOT)):
                        xpiece(lo, hi)

                def mk_psg():
                    return pspool.tile([P, 512], mybir.dt.float32, name="psg", tag="psg")

                def main_aux(m, ps):
                    mm(ps[:, :C], w_sb[:, layout["main"][m]:layout["main"][m] + P],
                       xt[:, m, :], start=True, stop=False)

                def aux(m, ps):
                    mm(ps[:, :C], w_sb[:, layout["aux"][m]:layout["aux"][m] + P],
                       xt[:, (m + 1) % NSLOT, :], start=False, stop=False)

                def up(m, ps, xc_sb):
                    woff = layout["up"][m]
                    mm(ps[:, :C], w_sb[:, woff:woff + P], xc_sb, start=False, stop=True)

                ogs = {}

                def evac(m, ps, b=b):
                    g = m // OGRP
                    if m % OGRP == 0:
                        ogs[g] = opool.tile([P, OGRP, C], mybir.dt.bfloat16, name="og", tag="og")
                    og = ogs[g]
                    osl = og[:, m % OGRP:m % OGRP + 1, :]
                    if ACT_EVERY and m % ACT_EVERY == ACT_EVERY - 1:
                        nc.scalar.copy(osl, ps[:, None, :C])
                    else:
                        nc.vector.tensor_copy(osl, ps[:, None, :C])
                    last_og = (b == BPC - 1) and (m // OGRP == NT // OGRP - 1)
                    o0 = (m // OGRP) * OGRP
                    dst = lambda i, n: out_d[:, (b * NT + o0 + i) * C:(b * NT + o0 + i + n) * C]
                    if last_og and m % 2 == 1:
                        # tail: drain the final og in 2-tile pieces to overlap DMA
                        i = (m % OGRP) - 1
                        nc.sync.dma_start(out=dst(i, 2).rearrange("p (g n) -> p g n", g=2),
                                          in_=og[:, i:i + 2, :])
                    elif not last_og and m % OGRP == OGRP - 1:
                        nc.sync.dma_start(out=dst(0, OGRP).rearrange("p (g n) -> p g n", g=OGRP),
                                          in_=og)

                WARM = int(_os.environ.get("DOG_WARM", "6")) if b == 0 else 0
                warm_ps = {}
                for m in range(WARM):
                    warm_ps[m] = mk_psg()
                for m in range(WARM):
                    main_aux(m, warm_ps[m])
                for m in range(WARM):
                    aux(m, warm_ps[m])

                # ---- down matmuls -> xc psum ----
                xcp = xcps_pool.tile([P, 512], mybir.dt.float32, name="xcp", tag="xcp")
                nd = len(layout["down"])
                for i, (c, off) in enumerate(layout["down"]):
                    mm(xcp[:, :C], w_sb[:, off:off + P], xt[:, c, :],
                       start=(i == 0), stop=(i == nd - 1))
                # ---- xc evac ----
                xc_sb = xcsb_pool.tile([P, C], mybir.dt.bfloat16, name="xc", tag="xc")
                nc.vector.tensor_copy(xc_sb, xcp[:, :C])

                for m in range(WARM):
                    up(m, warm_ps[m], xc_sb)
                    evac(m, warm_ps[m])
                warm_ps = None

                # ---- remaining tiles, processed in pairs for LDW run-sharing ----
                for m0 in range(WARM, NT, 2):
                    pa, pb_ = mk_psg(), mk_psg()
                    main_aux(m0, pa); main_aux(m0 + 1, pb_)
                    aux(m0, pa); aux(m0 + 1, pb_)
                    up(m0, pa, xc_sb); up(m0 + 1, pb_, xc_sb)
                    evac(m0, pa); evac(m0 + 1, pb_)
    _dedupe_ldweights(nc)
    nc.compile()
    return nc


_CACHE = {}


def _get_state():
    if "nc" not in _CACHE:
        w_np, layout = _build_weights()
        _CACHE["w"] = w_np.astype(BF16)
        _CACHE["nc"] = _build_program(w_np.shape[1], layout)
    return _CACHE["nc"], _CACHE["w"]


def _pack_x(xs):
    """xs [BPC, L, C] f32 -> [128, BPC*NSLOT*C] bf16 (shifted chunk layout;
    chunk 32's 17 rows occupy chunk 0's zero partitions 0..16)."""
    xb = xs.astype(BF16)
    chunks = np.zeros((BPC, NSLOT, P, C), BF16)
    chunks[:, 1:32].reshape(BPC, -1, C)[...] = xb[:, 111:4079].reshape(BPC, -1, C)
    chunks[:, 0, SH:, :] = xb[:, :111, :]
    chunks[:, 0, :SH, :] = xb[:, 4079:, :]
    return np.ascontiguousarray(chunks.transpose(2, 0, 1, 3).reshape(P, -1))


def run(x, **spmd_kwargs):
    x = np.asarray(x)
    nc, w_np = _get_state()
    in_maps = [{"x": _pack_x(x[core * BPC:(core + 1) * BPC]), "w": w_np}
               for core in range(N_CORES)]
    res = run_bass_kernel_spmd(nc, in_maps, list(range(N_CORES)), **spmd_kwargs)
    outs = []
    for i in range(N_CORES):
        o = np.asarray(res.results[i]["out"]).reshape(P, BPC, NT, C)
        outs.append(o.transpose(1, 2, 0, 3).reshape(BPC, L, C))
    return np.concatenate(outs, axis=0).astype(np.float32), res


def kernel(x):
    return run(x)[0]


# revision 48
# speedup vs baseline: 1.0201x; 1.0013x over previous
"""Trainium2 Bass kernel for nn_DoG_Seasonal: depthwise Difference-of-Gaussians
1-D conv along L (sigma 4.2 / 96, reflect padding), y = x*k1 - x*k2.

Multirate scheme (positions on partitions, channels on the free dim):
  - narrow path: k1 (35 taps) exact, on a chunk grid shifted by -17 so each
    output tile of 128 positions needs exactly 2 matmuls: main (chunk m,
    K=128 M=128) + aux (chunk m+1; weight columns 0..93 zero).
  - wide path: k2 (sigma=96) is low-bandwidth -> sketch xc = D x with 128
    overlapping triangular bins (width 64, spacing 32; piecewise-linear row
    space) via 33 "down" matmuls per batch (interior blocks are column-shifted
    slices of ONE [128,248] weight strip), then per tile one "up" matmul
    y2_tile = A_m @ xc (K=128, A_m least-squares fitted per tile against the
    exact reflect k2 operator; minus sign folded in) accumulated into the same
    PSUM bank as the narrow matmuls.

130 matmuls/batch (32 main + 32 aux + 33 down + 32 up + batch-0 warmup
ordering) vs 160 for the dense banded-Toeplitz formulation; evacuation stays
one PSUM->SBUF copy per tile (alternating DVE/ACT). All matmuls are chained
with scheduling-only deps so PSUM accumulate groups execute start-first on
reused banks. Head DMAs are interleaved (narrow w, x pieces, down strip, up
blocks) to minimize the PE cold start; the final output group drains in
2-tile DMA pieces to shorten the tail.

Sharding: data-parallel over batch - 32 batches / 8 cores, no cross-core
communication. Host packs x into a partition-major shifted-chunk DRAM layout
([128, BPC*33*321] bf16) so every DMA is contiguous per partition line;
output is returned the same way and un-packed on host.
"""

import numpy as np
import ml_dtypes

import concourse.bacc as bacc
import concourse.mybir as mybir
import concourse.tile as tile
from concourse.bass_utils import run_bass_kernel_spmd

# ---- problem constants ----
B, L, C = 32, 4096, 321
N_CORES = 8
BPC = B // N_CORES
P = 128
NT = L // P           # 32 output tiles per batch
NCH = NT + 1          # 33 shifted chunks per batch (conceptual)
NSLOT = NT            # 32 x slots: chunk 32's 17 rows ride in chunk 0's padding
SH = 17               # chunk grid shift
SIGMA1, SIGMA2, TRUNCATE = 4.2, 96.0, 4.0

import os as _os
OGRP = int(_os.environ.get("DOG_OGRP", "32"))     # out tiles per out-DMA
ACT_EVERY = int(_os.environ.get("DOG_ACT_EVERY", "2"))  # every k-th evac on ScalarE

BF16 = ml_dtypes.bfloat16


# ---------------- host-side weight construction ----------------

def _gauss(sigma):
    r = int(TRUNCATE * sigma + 0.5)
    t = np.arange(-r, r + 1, dtype=np.float64)
    k = np.exp(-0.5 * (t / sigma) ** 2)
    return k / k.sum()


def _chunk_range(c):
    return max(0, 128 * c - SH), min(L, 128 * c - SH + P)


def _reflect(u):
    v = np.abs(u)
    return np.where(v > L - 1, 2 * (L - 1) - v, v)


def _conv_rows(k, rows):
    r = (len(k) - 1) // 2
    W = np.zeros((len(rows), L))
    t = np.arange(-r, r + 1)
    for i, o in enumerate(rows):
        np.add.at(W[i], _reflect(o + t), k)
    return W


def _build_narrow():
    """Per tile m: {chunk c: block [128,128]} (lhsT rows=chunk partitions, cols=outs)."""
    k1 = _gauss(SIGMA1)
    r1 = (len(k1) - 1) // 2
    t = np.arange(-r1, r1 + 1)
    out = []
    for m in range(NT):
        W_full = np.zeros((L, P))
        for j in range(P):
            np.add.at(W_full[:, j], _reflect(128 * m + j + t), k1)
        per = {}
        for c in range(NCH):
            s, e = _chunk_range(c)
            blk = W_full[s:e, :]
            if not np.any(blk):
                continue
            p0 = s - (128 * c - SH)
            full = np.zeros((P, P))
            full[p0:p0 + (e - s), :] = blk
            per[c] = full
        assert set(per) <= {m, m + 1}, (m, sorted(per))
        out.append(per)
    return out


def _build_D():
    """[128 bins, L] overlapping triangular bins (width 64, spacing 32)."""
    D = np.zeros((P, L))
    u = np.arange(L, dtype=np.float64)
    ue = np.clip(u, -1.5, 32.0 * (P - 1) - 1.5)
    for k in range(P):
        D[k] = np.maximum(0.0, 1.0 - np.abs(ue - (32 * k - 1.5)) / 32.0)
    D /= D.sum(axis=0)[None, :]
    D *= 32.0
    return D


def _up_window(m):
    return min(max(4 * m - 14, 0), 96)


def _build_up(D, lam=1e-6):
    """Per tile m: A_m [128, 128] (rows=bins, zero outside the 32-bin window);
    minus sign folded. K=128 contraction of the full xc avoids partition-base
    gymnastics at the cost of per-tile weight blocks."""
    k2 = _gauss(SIGMA2)
    A_int = None
    out = []
    for m in range(NT):
        s = _up_window(m)
        if 4 <= m <= 27 and A_int is not None:
            A32 = A_int
        else:
            K2rows = _conv_rows(k2, np.arange(128 * m, 128 * m + P))
            Dw = D[s:s + 32]
            G = Dw @ Dw.T
            A32 = -np.linalg.solve(G + lam * np.trace(G) / 32 * np.eye(32),
                                   Dw @ K2rows.T)
            if 4 <= m <= 27:
                A_int = A32
        full = np.zeros((P, P))
        full[s:s + 32, :] = A32
        out.append(full)
    return out


def _build_down(D):
    """Down matmuls: list of (chunk c, block [128, 128]). Full M=128 out (zero
    weight columns outside the chunk's ~6-bin support) so the PSUM out AP
    never needs a partition offset."""
    mms = []
    for c in range(NCH):
        s, e = _chunk_range(c)
        p0 = s - (128 * c - SH)
        blk = np.zeros((P, P))           # [chunk partition, bin]
        blk[p0:p0 + (e - s), :] = D[:, s:e].T
        mms.append((c, blk))
    return mms


def _build_weights():
    """Returns (w_np [128, WCOLS] f32, layout dict)."""
    narrow = _build_narrow()
    D = _build_D()
    ups = _build_up(D)
    downs = _build_down(D)

    cols = []          # list of (width, array [128, width])
    uniq = {}

    def intern(arr):
        key = arr.astype(np.float32).tobytes()
        if key not in uniq:
            uniq[key] = (len(uniq), sum(w for w, _ in cols))
            cols.append((arr.shape[1], arr.astype(np.float32)))
        return uniq[key][1]   # column offset

    layout = {"main": [], "aux": [], "down": [], "up": []}
    for m in range(NT):
        layout["main"].append(intern(narrow[m][m]))
        layout["aux"].append(intern(narrow[m][m + 1]))
    # interior down blocks are column-shifted slices of one [128, 248] strip
    dmap = dict(downs)
    # chunk 32 (17 rows) is packed into chunk-0's zero partitions 0..16, so
    # its down block merges into chunk 0's (disjoint nonzero rows)
    assert not np.any(dmap[0][0:17]) and not np.any(dmap[32][17:])
    dmap[0] = dmap[0] + dmap[32]
    del dmap[32]
    Z = 124
    F = np.zeros((P, 248))
    F[:, 60:188] = dmap[16]
    f_off = intern(F)
    for c in range(NCH - 1):
        lo = Z - 4 * c
        if 0 <= lo and lo + P <= 248 and np.allclose(F[:, lo:lo + P], dmap[c], atol=1e-12):
            layout["down"].append((c, f_off + lo))
        else:
            layout["down"].append((c, intern(dmap[c])))
    for m in range(NT):
        layout["up"].append(intern(ups[m]))

    wcols = sum(w for w, _ in cols)
    w_np = np.zeros((P, wcols), np.float32)
    off = 0
    for w, arr in cols:
        w_np[:, off:off + w] = arr
        off += w
    return w_np, layout



def _dedupe_ldweights(nc):
    """Remove redundant InstLdweights: consecutive (in PE program order) loads
    of the identical weights AP need only the first load (~60-107 ns/LDW on HW,
    unmodeled by the cost sim)."""
    removed = 0
    for bi, blk in enumerate(nc.main_func.blocks):
        last_key = None
        new = []
        changed = False
        for inst in blk.instructions:
            nm = type(inst).__name__
            if nm == "InstLdweights":
                key = str(inst.ins[0])
                si = inst.sync_info
                clean = si is None or (len(si.on_wait) == 0 and len(si.on_update) == 0)
                if key == last_key and clean:
                    removed += 1
                    changed = True
                    continue
                last_key = key
            elif nm == "InstMatmult":
                pass
            elif getattr(inst, "engine", None) == mybir.EngineType.PE:
                last_key = None
            new.append(inst)
        if changed:
            blk.instructions = new
    return removed


# ---------------- device program ----------------

def _build_program(wcols, layout):
    _PREV = [None]
    nc = bacc.Bacc(None, target_bir_lowering=False)

    def mm(*args, **kwargs):
        """matmul chained in program order (scheduling-only dep) so PSUM
        accumulate groups execute start-first on reused banks and equal-weight
        runs stay consecutive for LDWEIGHTS."""
        r = nc.tensor.matmul(*args, **kwargs)
        if _PREV[0] is not None:
            tile.add_dep_helper(r.ins, _PREV[0].ins, sync=False, reason="pe order")
        _PREV[0] = r
        return r

    x_d = nc.declare_dram_parameter("x", [P, BPC * NSLOT * C], mybir.dt.bfloat16, isOutput=False)
    w_d = nc.declare_dram_parameter("w", [P, wcols], mybir.dt.bfloat16, isOutput=False)
    out_d = nc.declare_dram_parameter("out", [P, BPC * NT * C], mybir.dt.bfloat16, isOutput=True)

    with tile.TileContext(nc) as tc:
        with (
            tc.tile_pool(name="wpool", bufs=1) as wpool,
            tc.tile_pool(name="xpool", bufs=int(_os.environ.get("DOG_XBUFS", "2"))) as xpool,
            tc.tile_pool(name="xcsb", bufs=2) as xcsb_pool,
            tc.tile_pool(name="opool", bufs=2) as opool,
            tc.tile_pool(name="psum", bufs=int(_os.environ.get("DOG_PSBUFS", "7")), space="PSUM") as pspool,
            tc.tile_pool(name="xcps", bufs=int(_os.environ.get("DOG_XCPSBUFS", "1")), space="PSUM") as xcps_pool,
        ):
            w_sb = wpool.tile([P, wcols], mybir.dt.bfloat16)
            # piece boundaries: narrow blocks | down blocks | up blocks
            wn = min(o for _, o in layout["down"])
            wu = min(layout["up"])
            wu_cuts = [wu, layout["up"][8], layout["up"][16], layout["up"][24], wcols]
            # up blocks have only 32 nonzero rows; zero the region once (DVE is
            # idle at the head) and DMA just the nonzero row-bands per quad
            nc.vector.memset(w_sb[:, wu:], 0.0)
            wu_rows = []
            for q in range(4):
                r0 = _up_window(8 * q)
                r1 = _up_window(8 * q + 7) + 32
                wu_rows.append((r0, r1))

            for b in range(BPC):
                xt = xpool.tile([P, NSLOT, C], mybir.dt.bfloat16, name="xt", tag="xt")

                def xpiece(lo, hi, b=b, xt=xt):
                    nc.sync.dma_start(out=xt[:, lo:hi, :],
                                      in_=x_d[:, (b * NSLOT + lo) * C:(b * NSLOT + hi) * C]
                                      .rearrange("p (c n) -> p c n", c=hi - lo))

                if b == 0:
                    # head interleave: wn, x1, wd, x2, x3, wu1, x4, wu2..4
                    nc.sync.dma_start(out=w_sb[:, :wn], in_=w_d[:, :wn])
                    xpiece(0, 3)
                    xpiece(3, 7)
                    nc.sync.dma_start(out=w_sb[:, wn:wu], in_=w_d[:, wn:wu])
                    xpiece(7, 11)
                    xpiece(11, 15)
                    xpiece(15, 19)
                    xpiece(19, 23)
                    xpiece(23, 27)
                    xpiece(27, NSLOT)
                    for i in range(4):
                        r0, r1 = wu_rows[i]
                        nc.sync.dma_start(
                            out=w_sb[r0:r1, wu_cuts[i]:wu_cuts[i + 1]],
                            in_=w_d[r0:r1, wu_cuts[i]:wu_cuts[i + 1]])
                else:
                    for lo, hi in ((0, 17), (17, NSLOT)):
                        xpiece(lo, hi)

                def mk_psg():
                    return pspool.tile([P, 512], mybir.dt.float32, name="psg", tag="psg")

                def main_aux(m, ps):
                    mm(ps[:, :C], w_sb[:, layout["main"][m]:layout["main"][m] + P],
                       xt[:, m, :], start=True, stop=False)

                def aux(m, ps):
                    mm(ps[:, :C], w_sb[:, layout["aux"][m]:layout["aux"][m] + P],
                       xt[:, (m + 1) % NSLOT, :], start=False, stop=False)

                def up(m, ps, xc_sb):
                    woff = layout["up"][m]
                    mm(ps[:, :C], w_sb[:, woff:woff + P], xc_sb, start=False, stop=True)

                ogs = {}

                def evac(m, ps, b=b):
                    g = m // OGRP
                    if m % OGRP == 0:
                        ogs[g] = opool.tile([P, OGRP, C], mybir.dt.bfloat16, name="og", tag="og")
                    og = ogs[g]
                    osl = og[:, m % OGRP:m % OGRP + 1, :]
                    if ACT_EVERY and m % ACT_EVERY == ACT_EVERY - 1:
                        nc.scalar.copy(osl, ps[:, None, :C])
                    else:
                        nc.vector.tensor_copy(osl, ps[:, None, :C])
                    last_og = (b == BPC - 1) and (m // OGRP == NT // OGRP - 1)
                    o0 = (m // OGRP) * OGRP
                    dst = lambda i, n: out_d[:, (b * NT + o0 + i) * C:(b * NT + o0 + i + n) * C]
                    if last_og and m % 2 == 1:
                        # tail: drain the final og in 2-tile pieces to overlap DMA
                        i = (m % OGRP) - 1
                        nc.sync.dma_start(out=dst(i, 2).rearrange("p (g n) -> p g n", g=2),
                                          in_=og[:, i:i + 2, :])
                    elif not last_og and m % OGRP == OGRP - 1:
                        nc.sync.dma_start(out=dst(0, OGRP).rearrange("p (g n) -> p g n", g=OGRP),
                                          in_=og)

                WARM = int(_os.environ.get("DOG_WARM", "6")) if b == 0 else 0
                warm_ps = {}
                for m in range(WARM):
                    warm_ps[m] = mk_psg()
                for m in range(WARM):
                    main_aux(m, warm_ps[m])
                for m in range(WARM):
                    aux(m, warm_ps[m])

                # ---- down matmuls -> xc psum ----
                xcp = xcps_pool.tile([P, 512], mybir.dt.float32, name="xcp", tag="xcp")
                nd = len(layout["down"])
                for i, (c, off) in enumerate(layout["down"]):
                    mm(xcp[:, :C], w_sb[:, off:off + P], xt[:, c, :],
                       start=(i == 0), stop=(i == nd - 1))
                # ---- xc evac ----
                xc_sb = xcsb_pool.tile([P, C], mybir.dt.bfloat16, name="xc", tag="xc")
                nc.vector.tensor_copy(xc_sb, xcp[:, :C])

                for m in range(WARM):
                    up(m, warm_ps[m], xc_sb)
                    evac(m, warm_ps[m])
                warm_ps = None

                # ---- remaining tiles, processed in pairs for LDW run-sharing ----
                for m0 in range(WARM, NT, 2):
                    pa, pb_ = mk_psg(), mk_psg()
                    main_aux(m0, pa); main_aux(m0 + 1, pb_)
                    aux(m0, pa); aux(m0 + 1, pb_)
                    up(m0, pa, xc_sb); up(m0 + 1, pb_, xc_sb)
                    evac(m0, pa); evac(m0 + 1, pb_)
    _dedupe_ldweights(nc)
    nc.compile()
    return nc


_CACHE = {}


def _get_state():
    if "nc" not in _CACHE:
        w_np, layout = _build_weights()
        _CACHE["w"] = w_np.astype(BF16)
        _CACHE["nc"] = _build_program(w_np.shape[1], layout)
    return _CACHE["nc"], _CACHE["w"]


def _pack_x(xs):
    """xs [BPC, L, C] f32 -> [128, BPC*NSLOT*C] bf16 (shifted chunk layout;
    chunk 32's 17 rows occupy chunk 0's zero partitions 0..16)."""
    xb = xs.astype(BF16)
    chunks = np.zeros((BPC, NSLOT, P, C), BF16)
    chunks[:, 1:32].reshape(BPC, -1, C)[...] = xb[:, 111:4079].reshape(BPC, -1, C)
    chunks[:, 0, SH:, :] = xb[:, :111, :]
    chunks[:, 0, :SH, :] = xb[:, 4079:, :]
    return np.ascontiguousarray(chunks.transpose(2, 0, 1, 3).reshape(P, -1))


def run(x, **spmd_kwargs):
    x = np.asarray(x)
    nc, w_np = _get_state()
    in_maps = [{"x": _pack_x(x[core * BPC:(core + 1) * BPC]), "w": w_np}
               for core in range(N_CORES)]
    res = run_bass_kernel_spmd(nc, in_maps, list(range(N_CORES)), **spmd_kwargs)
    outs = []
    for i in range(N_CORES):
        o = np.asarray(res.results[i]["out"]).reshape(P, BPC, NT, C)
        outs.append(o.transpose(1, 2, 0, 3).reshape(BPC, L, C))
    return np.concatenate(outs, axis=0).astype(np.float32), res


def kernel(x):
    return run(x)[0]
